# revision 15
# baseline (speedup 1.0000x reference)
"""Gaussian vector-quantizer (Gumbel-softmax VQ) Bass kernel for 8 TRN2 cores.

Data-parallel over the flattened token axis N = B*T = 8*4096 = 32768.
Each core handles one batch element (4096 tokens). The [1024, 256]
codebook is replicated; mean_prob partial sums are reduced on host.

Math notes (vs reference.py):
  - logits = -(||z||^2 + ||e||^2 - 2 z.e) * pq.  The ||z||^2 term is a
    per-row constant, which softmax / log_softmax / gumbel-softmax are
    all invariant to, so it is dropped entirely.
  - We compute s = (2*pq*book^T) @ z - pq*||e||^2 directly in PSUM via
    matmul, with the per-codeword bias term folded in as an extra
    rank-1 accumulation (ones row x bias row).
  - prob = exp(s) / sum(exp(s))            (s <= ~+6, no overflow)
  - log_prob = s + ln(1/sum(exp(s)))
  - enc = softmax(2*(log_prob + g)), g = -ln(-ln(u+eps)+eps)
    (log_prob differs from logits by a row constant -> same softmax)
  - z_q^T = book^T @ enc^T  accumulated per 128-wide k chunk; enc^T is
    produced with PE transposes.
  - mean_prob partial: DVE accumulator over prob tiles + one final
    cross-partition reduce on GpSimd; summed across cores on host.

Matmuls run in float32r (tfloat32) for 4x PE throughput; operands that
feed matmuls are declared float32r end-to-end (walrus BIR rule).
"""

import numpy as np

import concourse.bacc as bacc
import concourse.bass as bass
import concourse.bass_isa as bass_isa
import concourse.tile as tile
from concourse import mybir

F32 = mybir.dt.float32
F32R = mybir.dt.float32r

B = 8
C = 256
T = 4096
K = 1024
N_CORES = 8
TOK_TILE = 128
N_TILES = T // TOK_TILE          # 32 per core
GROUP = 2                        # tiles per GEMM2 group (moving dim 256)
EPS = 1e-10

_NC_CACHE = {}


def build_nc(use_f32r=True):
    nc = bacc.Bacc("TRN2", target_bir_lowering=False, debug=False,
                   num_devices=N_CORES)
    MDT = F32R if use_f32r else F32

    z_d = nc.dram_tensor("z", [C, T], MDT, kind="ExternalInput")
    u_d = nc.dram_tensor("u", [T, K], F32, kind="ExternalInput")
    bt2_d = nc.dram_tensor("bt2", [C, K], MDT, kind="ExternalInput")  # 2*pq * book.T
    bn_d = nc.dram_tensor("bn", [K, C], MDT, kind="ExternalInput")    # book
    bb_d = nc.dram_tensor("bb", [1, K], MDT, kind="ExternalInput")    # -pq * ||e||^2
    id_d = nc.dram_tensor("ident", [128, 128], F32, kind="ExternalInput")
    ones_d = nc.dram_tensor("ones", [1, 128], MDT, kind="ExternalInput")

    prob_d = nc.dram_tensor("prob", [T, K], F32, kind="ExternalOutput")
    logp_d = nc.dram_tensor("logp", [T, K], F32, kind="ExternalOutput")
    zq_d = nc.dram_tensor("zq", [C, T], F32, kind="ExternalOutput")
    mp_d = nc.dram_tensor("mp", [1, K], F32, kind="ExternalOutput")

    with tile.TileContext(nc) as tc:
        with (
            tc.tile_pool(name="singles", bufs=1) as singles,
            tc.tile_pool(name="zp", bufs=3) as zp,
            tc.tile_pool(name="up", bufs=3) as up,
            tc.tile_pool(name="work", bufs=2) as work,
            tc.tile_pool(name="small", bufs=4) as small,
            tc.tile_pool(name="etp", bufs=2) as etp,
            tc.tile_pool(name="zqp", bufs=2) as zqp,
            tc.tile_pool(name="ps_s", bufs=2, space="PSUM") as ps_s,
            tc.tile_pool(name="ps_tr", bufs=2, space="PSUM") as ps_tr,
            tc.tile_pool(name="ps_zq", bufs=2, space="PSUM") as ps_zq,
        ):
            # ---- preload constants ----
            btile = singles.tile([128, 2, K], MDT)       # bt2 rearranged
            nc.sync.dma_start(
                out=btile[:], in_=bt2_d[:, :].rearrange("(cc p) k -> p cc k", p=128)
            )
            bntile = singles.tile([128, 8, C], MDT)      # book rearranged
            nc.sync.dma_start(
                out=bntile[:], in_=bn_d[:, :].rearrange("(kc p) c -> p kc c", p=128)
            )
            bbt = singles.tile([1, K], MDT)
            nc.sync.dma_start(out=bbt[:], in_=bb_d[:, :])
            idt = singles.tile([128, 128], F32)
            nc.sync.dma_start(out=idt[:], in_=id_d[:, :])
            ones1 = singles.tile([1, 128], MDT)
            nc.sync.dma_start(out=ones1[:], in_=ones_d[:, :])
            epst = singles.tile([128, 1], F32)
            nc.vector.memset(epst[:], EPS)
            acc = singles.tile([128, K], F32, tag="acc")
            nc.vector.memset(acc[:], 0.0)

            zre = z_d[:, :].rearrange("(cc p) t -> p cc t", p=128)
            zqre = zq_d[:, :].rearrange("(cc p) t -> p cc t", p=128)

            for g in range(N_TILES // GROUP):
                encT = etp.tile([128, 8, GROUP * TOK_TILE], MDT)
                for i in range(GROUP):
                    t = g * GROUP + i
                    t0 = t * TOK_TILE

                    zt = zp.tile([128, 2, TOK_TILE], MDT)
                    nc.sync.dma_start(out=zt[:], in_=zre[:, :, t0:t0 + TOK_TILE])
                    ut = up.tile([128, K], F32)
                    nc.sync.dma_start(out=ut[:], in_=u_d[t0:t0 + TOK_TILE, :])

                    # GEMM1: s = (2pq book^T) @ z - pq*||e||^2  -> logits'
                    s = ps_s.tile([128, K], F32)
                    for bank in range(2):
                        ks = slice(bank * 512, (bank + 1) * 512)
                        nc.tensor.matmul(
                            s[:, ks], zt[:, 0, :], btile[:, 0, ks],
                            start=True, stop=False)
                        nc.tensor.matmul(
                            s[:, ks], zt[:, 1, :], btile[:, 1, ks],
                            start=False, stop=False)
                        nc.tensor.matmul(
                            s[:, ks], ones1[:], bbt[:, ks],
                            start=False, stop=True)

                    # softmax over K (no max-shift needed: s <= ~+6)
                    e = work.tile([128, K], F32, tag="e")
                    ssum = small.tile([128, 1], F32, tag="ssum")
                    nc.scalar.activation(
                        out=e[:], in_=s[:], func=mybir.ActivationFunctionType.Exp,
                        accum_out=ssum[:])
                    r = small.tile([128, 1], F32, tag="r")
                    nc.vector.reciprocal(out=r[:], in_=ssum[:])
                    prob = work.tile([128, K], F32, tag="prob")
                    nc.gpsimd.tensor_scalar_mul(prob[:], e[:], r[:])
                    nc.sync.dma_start(out=prob_d[t0:t0 + TOK_TILE, :], in_=prob[:])

                    # mean_prob accumulator
                    nc.vector.tensor_add(acc[:], acc[:], prob[:])

                    # log_prob = s + ln(1/ssum)
                    lsn = small.tile([128, 1], F32, tag="lsn")
                    nc.scalar.activation(
                        out=lsn[:], in_=r[:], func=mybir.ActivationFunctionType.Ln)
                    lp = work.tile([128, K], F32, tag="lp")
                    nc.vector.tensor_scalar_add(lp[:], s[:], lsn[:])
                    nc.sync.dma_start(out=logp_d[t0:t0 + TOK_TILE, :], in_=lp[:])

                    # gumbel: g = -l2, l2 = ln(eps - ln(u + eps))
                    l1 = work.tile([128, K], F32, tag="l1")
                    nc.scalar.activation(
                        out=l1[:], in_=ut[:], func=mybir.ActivationFunctionType.Ln,
                        bias=epst[:], scale=1.0)
                    l2 = work.tile([128, K], F32, tag="l2")
                    nc.scalar.activation(
                        out=l2[:], in_=l1[:], func=mybir.ActivationFunctionType.Ln,
                        bias=epst[:], scale=-1.0)

                    # arg = lp - l2 = log_prob + g  (no max-shift needed:
                    # 2*arg is within [-98, +34], safe for fp32 exp)
                    arg = work.tile([128, K], F32, tag="arg")
                    nc.vector.tensor_sub(arg[:], lp[:], l2[:])

                    e2 = work.tile([128, K], F32, tag="e2")
                    ssum2 = small.tile([128, 1], F32, tag="ssum2")
                    nc.scalar.activation(
                        out=e2[:], in_=arg[:], func=mybir.ActivationFunctionType.Exp,
                        scale=2.0, accum_out=ssum2[:])
                    r2 = small.tile([128, 1], F32, tag="r2")
                    nc.vector.reciprocal(out=r2[:], in_=ssum2[:])
                    enc = work.tile([128, K], F32, tag="enc")
                    nc.gpsimd.tensor_scalar_mul(enc[:], e2[:], r2[:])

                    # transpose enc into encT[:, kc, i*128:(i+1)*128]
                    for half in range(2):
                        tr = ps_tr.tile([128, 4, 128], F32, tag="tr")
                        for j in range(4):
                            kc = half * 4 + j
                            nc.tensor.transpose(
                                tr[:, j, :], enc[:, kc * 128:(kc + 1) * 128],
                                idt[:])
                        nc.vector.tensor_copy(
                            out=encT[:, half * 4:(half + 1) * 4,
                                     i * TOK_TILE:(i + 1) * TOK_TILE],
                            in_=tr[:])

                # GEMM2: zq^T[c, tok] = sum_k book[k, c] * encT[k, tok]
                zq = ps_zq.tile([128, 2, GROUP * TOK_TILE], F32)
                for cc in range(2):
                    cs = slice(cc * 128, (cc + 1) * 128)
                    for kc in range(8):
                        nc.tensor.matmul(
                            zq[:, cc, :], bntile[:, kc, cs], encT[:, kc, :],
                            start=(kc == 0), stop=(kc == 7))
                zqs = zqp.tile([128, 2, GROUP * TOK_TILE], F32)
                nc.vector.tensor_copy(out=zqs[:], in_=zq[:])
                gt0 = g * GROUP * TOK_TILE
                nc.sync.dma_start(
                    out=zqre[:, :, gt0:gt0 + GROUP * TOK_TILE], in_=zqs[:])

            # final cross-partition reduce of the prob accumulator
            mps = singles.tile([128, K], F32, tag="mps")
            nc.gpsimd.partition_all_reduce(
                mps[:], acc[:], channels=128, reduce_op=bass_isa.ReduceOp.add)
            nc.sync.dma_start(out=mp_d[:, :], in_=mps[0:1, :])

    nc.compile()
    return nc


def get_nc(use_f32r=True):
    key = bool(use_f32r)
    if key not in _NC_CACHE:
        _NC_CACHE[key] = build_nc(use_f32r=key)
    return _NC_CACHE[key]


def make_in_maps(z, book, log_param_q, u):
    z = np.ascontiguousarray(np.asarray(z, dtype=np.float32))
    book = np.ascontiguousarray(np.asarray(book, dtype=np.float32))
    u = np.ascontiguousarray(np.asarray(u, dtype=np.float32))
    lpq = np.float32(np.asarray(log_param_q, dtype=np.float32))

    param_q = np.float32(1.0) + np.exp(lpq, dtype=np.float32)
    pq = np.float32(0.5) / np.maximum(param_q, np.float32(1e-10))

    bt2 = np.ascontiguousarray((np.float32(2.0) * pq) * book.T)
    bb = (-pq) * np.sum(book * book, axis=1, dtype=np.float32)
    bb = np.ascontiguousarray(bb.reshape(1, K).astype(np.float32))
    ident = np.ascontiguousarray(np.eye(128, dtype=np.float32))

    n_loc = T
    in_maps = []
    for c in range(N_CORES):
        in_maps.append({
            "z": np.ascontiguousarray(z[c]),
            "u": np.ascontiguousarray(u[c * n_loc:(c + 1) * n_loc]),
            "bt2": bt2,
            "bn": book,
            "bb": bb,
            "ident": ident,
            "ones": np.ones((1, 128), dtype=np.float32),
        })
    return in_maps, pq


def assemble(results, pq):
    z_q = np.empty((B, C, T), dtype=np.float32)
    prob = np.empty((B * T, K), dtype=np.float32)
    log_prob = np.empty((B * T, K), dtype=np.float32)
    mp_acc = np.zeros((K,), dtype=np.float32)
    for c in range(N_CORES):
        rc = results[c]
        z_q[c] = rc["zq"]
        prob[c * T:(c + 1) * T] = rc["prob"]
        log_prob[c * T:(c + 1) * T] = rc["logp"]
        mp_acc += rc["mp"].reshape(K)
    mean_prob = (mp_acc / np.float32(B * T)).astype(np.float32)
    precision_q = np.float32(pq)
    return z_q, precision_q, prob, log_prob, mean_prob


def kernel(z, book, log_param_q, u, is_train=1, **_kwargs):
    from concourse.bass_utils import run_bass_kernel_spmd

    in_maps, pq = make_in_maps(z, book, log_param_q, u)
    nc = get_nc(use_f32r=True)
    res = run_bass_kernel_spmd(nc, in_maps, core_ids=list(range(N_CORES)))
    return assemble(res.results, pq)


# revision 18
# speedup vs baseline: 3.7829x; 3.7829x over previous
"""Gaussian vector-quantizer (Gumbel-softmax VQ) Bass kernel for 8 TRN2 cores.

Data-parallel over the flattened token axis N = B*T = 8*4096 = 32768.
Each core handles one batch element (4096 tokens). The [1024, 256]
codebook is replicated; mean_prob partial sums are reduced on host.

Math notes (vs reference.py):
  - logits = -(||z||^2 + ||e||^2 - 2 z.e) * pq.  The ||z||^2 term is a
    per-row constant, which softmax / log_softmax / gumbel-softmax are
    all invariant to, so it is dropped entirely.
  - We compute s = (2*pq*book^T) @ z - pq*||e||^2 directly in PSUM via
    matmul, with the per-codeword bias term folded in as an extra
    rank-1 accumulation (ones row x bias row).
  - prob = exp(s) / sum(exp(s))            (s <= ~+6, no overflow)
  - log_prob = s + ln(1/sum(exp(s)))
  - enc = softmax(2*(log_prob + g)), g = -ln(-ln(u+eps)+eps)
    (log_prob differs from logits by a row constant -> same softmax)
  - z_q^T = book^T @ enc^T  accumulated per 128-wide k chunk; enc^T is
    produced with PE transposes.
  - mean_prob partial: DVE accumulator over prob tiles + one final
    cross-partition reduce on GpSimd; summed across cores on host.

Matmuls run in float32r (tfloat32) for 4x PE throughput; operands that
feed matmuls are declared float32r end-to-end (walrus BIR rule).
"""

import numpy as np

import concourse.bacc as bacc
import concourse.bass as bass
import concourse.bass_isa as bass_isa
import concourse.tile as tile
from concourse import mybir

F32 = mybir.dt.float32
F32R = mybir.dt.float32r

B = 8
C = 256
T = 4096
K = 1024
N_CORES = 8
TOK_TILE = 128
N_TILES = T // TOK_TILE          # 32 per core
GROUP = 2                        # tiles per GEMM2 group (moving dim 256)
EPS = 1e-10

_NC_CACHE = {}


def _patch_act_tables():
    """Force the activation-table inserter to use the combined
    natural_log_exp_and_others set (covers Exp+Ln+Copy+Identity).  The
    default greedy choice thrashes between the exp-only and ln-only sets,
    emitting ~82 table loads (~2.7us each) instead of 1."""
    import concourse.bacc as bacc_mod
    import concourse.hw_specs as hw_specs
    if getattr(hw_specs, "_vq_act_patch", False):
        return
    orig = hw_specs.get_activation_tables

    def patched(arch):
        tabs = orig(arch)
        return {name: (fns if name == "natural_log_exp_and_others" else set())
                for name, fns in tabs.items()}

    hw_specs.get_activation_tables = patched
    bacc_mod.get_activation_tables = patched
    hw_specs._vq_act_patch = True


def build_nc(use_f32r=True):
    _patch_act_tables()
    nc = bacc.Bacc("TRN2", target_bir_lowering=False, debug=False,
                   num_devices=N_CORES)
    MDT = F32R if use_f32r else F32

    z_d = nc.dram_tensor("z", [C, T], MDT, kind="ExternalInput")
    u_d = nc.dram_tensor("u", [T, K], F32, kind="ExternalInput")
    bt2_d = nc.dram_tensor("bt2", [C, K], MDT, kind="ExternalInput")  # 2*pq * book.T
    bn_d = nc.dram_tensor("bn", [K, C], MDT, kind="ExternalInput")    # book
    bb_d = nc.dram_tensor("bb", [1, K], MDT, kind="ExternalInput")    # -pq * ||e||^2
    id_d = nc.dram_tensor("ident", [128, 128], F32, kind="ExternalInput")
    ones_d = nc.dram_tensor("ones", [1, 128], MDT, kind="ExternalInput")

    prob_d = nc.dram_tensor("prob", [T, K], F32, kind="ExternalOutput")
    logp_d = nc.dram_tensor("logp", [T, K], F32, kind="ExternalOutput")
    zq_d = nc.dram_tensor("zq", [C, T], F32, kind="ExternalOutput")
    mp_d = nc.dram_tensor("mp", [1, K], F32, kind="ExternalOutput")

    with tile.TileContext(nc) as tc:
        with (
            tc.tile_pool(name="singles", bufs=1) as singles,
            tc.tile_pool(name="zp", bufs=3) as zp,
            tc.tile_pool(name="up", bufs=3) as up,
            tc.tile_pool(name="work", bufs=2) as work,
            tc.tile_pool(name="small", bufs=4) as small,
            tc.tile_pool(name="etp", bufs=2) as etp,
            tc.tile_pool(name="zqp", bufs=2) as zqp,
            tc.tile_pool(name="ps_s", bufs=2, space="PSUM") as ps_s,
            tc.tile_pool(name="ps_tr", bufs=2, space="PSUM") as ps_tr,
            tc.tile_pool(name="ps_zq", bufs=2, space="PSUM") as ps_zq,
        ):
            # ---- preload constants ----
            btile = singles.tile([128, 2, K], MDT)       # bt2 rearranged
            nc.sync.dma_start(
                out=btile[:], in_=bt2_d[:, :].rearrange("(cc p) k -> p cc k", p=128)
            )
            bntile = singles.tile([128, 8, C], MDT)      # book rearranged
            nc.sync.dma_start(
                out=bntile[:], in_=bn_d[:, :].rearrange("(kc p) c -> p kc c", p=128)
            )
            bbt = singles.tile([1, K], MDT)
            nc.sync.dma_start(out=bbt[:], in_=bb_d[:, :])
            idt = singles.tile([128, 128], F32)
            nc.sync.dma_start(out=idt[:], in_=id_d[:, :])
            ones1 = singles.tile([1, 128], MDT)
            nc.sync.dma_start(out=ones1[:], in_=ones_d[:, :])
            epst = singles.tile([128, 1], F32)
            nc.vector.memset(epst[:], EPS)
            acc = singles.tile([128, K], F32, tag="acc")
            nc.vector.memset(acc[:], 0.0)

            zre = z_d[:, :].rearrange("(cc p) t -> p cc t", p=128)
            zqre = zq_d[:, :].rearrange("(cc p) t -> p cc t", p=128)

            for g in range(N_TILES // GROUP):
                encT = etp.tile([128, 8, GROUP * TOK_TILE], MDT)
                for i in range(GROUP):
                    t = g * GROUP + i
                    t0 = t * TOK_TILE

                    zt = zp.tile([128, 2, TOK_TILE], MDT)
                    nc.sync.dma_start(out=zt[:], in_=zre[:, :, t0:t0 + TOK_TILE])
                    ut = up.tile([128, K], F32)
                    nc.sync.dma_start(out=ut[:], in_=u_d[t0:t0 + TOK_TILE, :])

                    # GEMM1: s = (2pq book^T) @ z - pq*||e||^2  -> logits'
                    s = ps_s.tile([128, K], F32)
                    for bank in range(2):
                        ks = slice(bank * 512, (bank + 1) * 512)
                        nc.tensor.matmul(
                            s[:, ks], zt[:, 0, :], btile[:, 0, ks],
                            start=True, stop=False)
                        nc.tensor.matmul(
                            s[:, ks], zt[:, 1, :], btile[:, 1, ks],
                            start=False, stop=False)
                        nc.tensor.matmul(
                            s[:, ks], ones1[:], bbt[:, ks],
                            start=False, stop=True)

                    # softmax over K (no max-shift needed: s <= ~+6)
                    e = work.tile([128, K], F32, tag="e")
                    ssum = small.tile([128, 1], F32, tag="ssum")
                    nc.scalar.activation(
                        out=e[:], in_=s[:], func=mybir.ActivationFunctionType.Exp,
                        accum_out=ssum[:])
                    r = small.tile([128, 1], F32, tag="r")
                    nc.vector.reciprocal(out=r[:], in_=ssum[:])
                    prob = work.tile([128, K], F32, tag="prob")
                    nc.scalar.activation(
                        out=prob[:], in_=e[:],
                        func=mybir.ActivationFunctionType.Copy, scale=r[:])
                    nc.sync.dma_start(out=prob_d[t0:t0 + TOK_TILE, :], in_=prob[:])

                    # mean_prob accumulator
                    nc.vector.tensor_add(acc[:], acc[:], prob[:])

                    # log_prob = s + ln(1/ssum)
                    lsn = small.tile([128, 1], F32, tag="lsn")
                    nc.scalar.activation(
                        out=lsn[:], in_=r[:], func=mybir.ActivationFunctionType.Ln)
                    lp = work.tile([128, K], F32, tag="lp")
                    nc.vector.tensor_scalar_add(lp[:], s[:], lsn[:])
                    nc.sync.dma_start(out=logp_d[t0:t0 + TOK_TILE, :], in_=lp[:])

                    # gumbel: g = -l2, l2 = ln(eps - ln(u + eps))
                    l1 = work.tile([128, K], F32, tag="l1")
                    nc.scalar.activation(
                        out=l1[:], in_=ut[:], func=mybir.ActivationFunctionType.Ln,
                        bias=epst[:], scale=1.0)
                    l2 = work.tile([128, K], F32, tag="l2")
                    nc.scalar.activation(
                        out=l2[:], in_=l1[:], func=mybir.ActivationFunctionType.Ln,
                        bias=epst[:], scale=-1.0)

                    # arg = lp - l2 = log_prob + g  (no max-shift needed:
                    # 2*arg is within [-98, +34], safe for fp32 exp)
                    arg = work.tile([128, K], F32, tag="arg")
                    nc.vector.tensor_sub(arg[:], lp[:], l2[:])

                    e2 = work.tile([128, K], F32, tag="e2")
                    ssum2 = small.tile([128, 1], F32, tag="ssum2")
                    nc.scalar.activation(
                        out=e2[:], in_=arg[:], func=mybir.ActivationFunctionType.Exp,
                        scale=2.0, accum_out=ssum2[:])
                    r2 = small.tile([128, 1], F32, tag="r2")
                    nc.vector.reciprocal(out=r2[:], in_=ssum2[:])
                    enc = work.tile([128, K], F32, tag="enc")
                    nc.vector.tensor_scalar_mul(enc[:], e2[:], r2[:])

                    # transpose enc into encT[:, kc, i*128:(i+1)*128]
                    for half in range(2):
                        tr = ps_tr.tile([128, 4, 128], F32, tag="tr")
                        for j in range(4):
                            kc = half * 4 + j
                            nc.tensor.transpose(
                                tr[:, j, :], enc[:, kc * 128:(kc + 1) * 128],
                                idt[:])
                        nc.vector.tensor_copy(
                            out=encT[:, half * 4:(half + 1) * 4,
                                     i * TOK_TILE:(i + 1) * TOK_TILE],
                            in_=tr[:])

                # GEMM2: zq^T[c, tok] = sum_k book[k, c] * encT[k, tok]
                zq = ps_zq.tile([128, 2, GROUP * TOK_TILE], F32)
                for cc in range(2):
                    cs = slice(cc * 128, (cc + 1) * 128)
                    for kc in range(8):
                        nc.tensor.matmul(
                            zq[:, cc, :], bntile[:, kc, cs], encT[:, kc, :],
                            start=(kc == 0), stop=(kc == 7))
                zqs = zqp.tile([128, 2, GROUP * TOK_TILE], F32)
                nc.vector.tensor_copy(out=zqs[:], in_=zq[:])
                gt0 = g * GROUP * TOK_TILE
                nc.sync.dma_start(
                    out=zqre[:, :, gt0:gt0 + GROUP * TOK_TILE], in_=zqs[:])

            # final cross-partition reduce of the prob accumulator
            mps = singles.tile([128, K], F32, tag="mps")
            nc.gpsimd.partition_all_reduce(
                mps[:], acc[:], channels=128, reduce_op=bass_isa.ReduceOp.add)
            nc.sync.dma_start(out=mp_d[:, :], in_=mps[0:1, :])

    nc.compile()
    return nc


def get_nc(use_f32r=True):
    key = bool(use_f32r)
    if key not in _NC_CACHE:
        _NC_CACHE[key] = build_nc(use_f32r=key)
    return _NC_CACHE[key]


def make_in_maps(z, book, log_param_q, u):
    z = np.ascontiguousarray(np.asarray(z, dtype=np.float32))
    book = np.ascontiguousarray(np.asarray(book, dtype=np.float32))
    u = np.ascontiguousarray(np.asarray(u, dtype=np.float32))
    lpq = np.float32(np.asarray(log_param_q, dtype=np.float32))

    param_q = np.float32(1.0) + np.exp(lpq, dtype=np.float32)
    pq = np.float32(0.5) / np.maximum(param_q, np.float32(1e-10))

    bt2 = np.ascontiguousarray((np.float32(2.0) * pq) * book.T)
    bb = (-pq) * np.sum(book * book, axis=1, dtype=np.float32)
    bb = np.ascontiguousarray(bb.reshape(1, K).astype(np.float32))
    ident = np.ascontiguousarray(np.eye(128, dtype=np.float32))

    n_loc = T
    in_maps = []
    for c in range(N_CORES):
        in_maps.append({
            "z": np.ascontiguousarray(z[c]),
            "u": np.ascontiguousarray(u[c * n_loc:(c + 1) * n_loc]),
            "bt2": bt2,
            "bn": book,
            "bb": bb,
            "ident": ident,
            "ones": np.ones((1, 128), dtype=np.float32),
        })
    return in_maps, pq


def assemble(results, pq):
    z_q = np.empty((B, C, T), dtype=np.float32)
    prob = np.empty((B * T, K), dtype=np.float32)
    log_prob = np.empty((B * T, K), dtype=np.float32)
    mp_acc = np.zeros((K,), dtype=np.float32)
    for c in range(N_CORES):
        rc = results[c]
        z_q[c] = rc["zq"]
        prob[c * T:(c + 1) * T] = rc["prob"]
        log_prob[c * T:(c + 1) * T] = rc["logp"]
        mp_acc += rc["mp"].reshape(K)
    mean_prob = (mp_acc / np.float32(B * T)).astype(np.float32)
    precision_q = np.float32(pq)
    return z_q, precision_q, prob, log_prob, mean_prob


def kernel(z, book, log_param_q, u, is_train=1, **_kwargs):
    from concourse.bass_utils import run_bass_kernel_spmd

    in_maps, pq = make_in_maps(z, book, log_param_q, u)
    nc = get_nc(use_f32r=True)
    res = run_bass_kernel_spmd(nc, in_maps, core_ids=list(range(N_CORES)))
    return assemble(res.results, pq)
